# revision 1
# baseline (speedup 1.0000x reference)
"""Trainium2 Bass kernel for nn_Diffusion_15436112462451.

Strategy: pure data parallelism over the batch (2048 -> 8 cores x 256),
feature-major activations on-chip, fully unrolled 100-step loop.

Per step (per core):
  - 16 bf16 matmuls on PE: L1 uses split-precision weights (W = Whi + Wlo,
    two accumulating matmuls) with the per-step bias table fused into the
    stationary operand as two extra K-rows (rhs rows pinned to 1.0); L2/L3
    get their bias from a single K=4 rank-1 matmul with a 0/1-mask rhs that
    covers both 128-feature chunks of the [128,512] PSUM bank.
  - 3 sigmoid passes on ScalarE (one per hidden layer).
  - 2 fused custom-DVE passes per layer evaluate the exact-mish rational
    completion  mish(z) ~= z * QUAD(t) * CUBIC(t) + beta,  t = sigmoid(-az-d)^2
    (degree-5 minimax fit of tanh(softplus), max err 6.3e-5; beta is folded
    into the next layer's bias on the host).
  - The denoising x-update runs on small [16,256] DVE ops with per-step
    schedule scalars baked in as immediates.

The time-embedding MLP is batch-independent (the timestep is a scalar per
step), so its contribution is precomputed on the host into a [100,256] bias
table baked into the L1 stationary operand (w1ext). All noise is preloaded
to SBUF in the preamble; no per-step DMA. The execution environment is
latency-bound on the serial dependency chain, so the design minimizes both
instruction count and accumulation-group depth.
"""
import sys
import math
import re
import numpy as np

for _p in ('/opt/trn_rl_repo', '/root/.axon_site/_ro/trn_rl_repo'):
    if _p not in sys.path:
        sys.path.insert(0, _p)

import ml_dtypes
from contextlib import ExitStack
import concourse.bass as bass
from concourse import bacc
from concourse import mybir, tile, bass_utils, dve_ops
from concourse.dve_spec import Spec, Src0, Src1, C0, C1, C2, sq, maxx, minn

BF16 = ml_dtypes.bfloat16
NCORES = 8
BATCH = 2048
BPC = BATCH // NCORES          # 256 batch rows per core
T_STEPS = 100
STATE_DIM, ACTION_DIM, HIDDEN, TIME_DIM = 64, 16, 256, 32
KX = ACTION_DIM + STATE_DIM    # 80 rows of W1 used for [x; state]

# --- activation fit constants (deg-5 sigma-poly factorization) ---
A_S = 0.9990298806699722
D_S = -0.0005000143935776705
BETA = 4.708088756431602e-05
QA, QB, QC = -0.21302398380145082, 0.6455208072356895, -0.6201860532189531
MA, MB, MC = -0.9194163848641597, 1.5334239721923986, -1.6124382654378613


# ---------------------------------------------------------------- custom ops
def _register_op(name, spec):
    for op in dve_ops.OPS:
        if op.name == name:
            return op
    op = dve_ops.DveOp(name, spec, False, uops_sha={"v3": "?", "v4": "?"})
    dve_ops.OPS.append(op)
    dve_ops.CUSTOM_DVE_SPECS[name] = spec
    dve_ops._SUB_OPCODE_FOR_NAME[name] = (
        dve_ops._CUSTOM_DVE_ROW_BASE + len(dve_ops.OPS) - 1)
    for ver in ("v3", "v4"):
        try:
            op.compile(ver)
        except ValueError as e:
            op.uops_sha[ver] = re.search(
                r'uops_sha\["' + ver + r'"\]="([0-9a-f]+)"', str(e)).group(1)
        op.compile(ver)
    return op


_t = sq(Src0)
MISH_A = _register_op("MISH_A_DIFF15436", Spec(
    body=Src1 * ((_t * C0 + C1) * _t + C2),
    reference=lambda in0, in1, s0, s1, imm2:
        (in1 * ((s0 * in0.astype(np.float64) ** 2 + s1) * in0.astype(np.float64) ** 2 + imm2)).astype(np.float32),
))
_t2 = sq(Src0)
MISH_B = _register_op("MISH_B_DIFF15436", Spec(
    body=Src1 * ((((_t2 + C0) * _t2 + C1) * _t2) + C2),
    reference=lambda in0, in1, s0, s1, imm2:
        (in1 * ((((in0.astype(np.float64) ** 2 + s0) * in0.astype(np.float64) ** 2 + s1) * in0.astype(np.float64) ** 2) + imm2)).astype(np.float32),
))
PREOP = _register_op("PREOP_DIFF15436", Spec(
    body=Src0 * C2 + Src1 * C1 + C0,
    reference=lambda in0, in1, s0, s1, imm2:
        (in0 * imm2 + in1 * s1 + s0).astype(np.float32),
))
CLIPMULADD = _register_op("CLIPMULADD_DIFF15436", Spec(
    body=minn(maxx(Src0, C0), C1) * C2 + Src1,
    reference=lambda in0, in1, s0, s1, imm2:
        (np.minimum(np.maximum(in0, s0), s1) * imm2 + in1).astype(np.float32),
))


# ---------------------------------------------------------------- schedule
def _vp_schedule():
    t = np.arange(1, T_STEPS + 1, dtype=np.float64)
    b_max, b_min = 10.0, 0.1
    alpha = np.exp(-b_min / T_STEPS - 0.5 * (b_max - b_min) * (2 * t - 1) / T_STEPS ** 2)
    betas = 1.0 - alpha
    ac = np.cumprod(1.0 - betas)
    ac_prev = np.concatenate([[1.0], ac[:-1]])
    return {
        'c1': np.sqrt(1.0 / ac).astype(np.float32),
        'c2': np.sqrt(1.0 / ac - 1.0).astype(np.float32),
        'p1': (betas * np.sqrt(ac_prev) / (1.0 - ac)).astype(np.float32),
        'p2': ((1.0 - ac_prev) * np.sqrt(1.0 - betas) / (1.0 - ac)).astype(np.float32),
        'logvar': np.log(np.clip(betas * (1.0 - ac_prev) / (1.0 - ac), 1e-20, None)).astype(np.float32),
    }


def _mish64(v):
    return v * np.tanh(np.logaddexp(0.0, v))


# ---------------------------------------------------------------- bass build
_CACHE = {}


def _build(nsteps=T_STEPS, use_b23=True):
    if ('nc', nsteps, use_b23) in _CACHE:
        return _CACHE[('nc', nsteps, use_b23)]
    sched = _vp_schedule()
    c1s, c2s, p1s, p2s = sched['c1'], sched['c2'], sched['p1'], sched['p2']

    nc = bacc.Bacc("TRN2", target_bir_lowering=False, debug=False, num_devices=NCORES)
    f32 = mybir.dt.float32
    bf = mybir.dt.bfloat16

    def din(name, shape, dt=f32):
        return nc.dram_tensor(name, shape, dt, kind="ExternalInput").ap()

    d_state = din("state_t", [STATE_DIM + 2, BPC], bf)
    d_xinit = din("x_init_t", [ACTION_DIM, BPC])
    d_noise = din("noise_t", [T_STEPS, ACTION_DIM, BPC])
    d_w1x_lo = din("w1x_lo", [KX, HIDDEN], bf)
    d_w2_hi = din("w2_hi", [HIDDEN, HIDDEN], bf)
    d_w3_hi = din("w3_hi", [HIDDEN, HIDDEN], bf)
    d_w4_hi = din("w4_hi", [HIDDEN, ACTION_DIM], bf)
    d_w1ext = din("w1ext", [KX + 2, T_STEPS * 2 * 128], bf)
    d_b23 = din("b23_hl", [4, 2 * 128], bf)
    d_mask = din("mask4", [4, 2 * BPC], bf)
    d_xb = din("xb_t", [ACTION_DIM, T_STEPS])
    d_out = nc.dram_tensor("out_t", [ACTION_DIM, BPC], f32, kind="ExternalOutput").ap()

    with tile.TileContext(nc) as tc, ExitStack() as ctx:
        wp = ctx.enter_context(tc.tile_pool(name="weights", bufs=1))
        ap_ = ctx.enter_context(tc.tile_pool(name="acts", bufs=2))
        sp = ctx.enter_context(tc.tile_pool(name="small", bufs=2))
        np_ = ctx.enter_context(tc.tile_pool(name="noise", bufs=4))
        pp = ctx.enter_context(tc.tile_pool(name="psum", bufs=2, space="PSUM"))

        def wtile(shape, dt, nm, src):
            t = wp.tile(shape, dt, tag=nm, name=nm)
            nc.gpsimd.dma_start(t, src)
            return t

        w1ext = wtile([KX + 2, T_STEPS * 2 * 128], bf, "w1ext", d_w1ext)
        w1x_lo = wtile([KX, HIDDEN], bf, "w1x_lo", d_w1x_lo)
        w2 = {}
        w3 = {}
        w4 = {}
        for nm, dhi, dst in (("w2", d_w2_hi, w2), ("w3", d_w3_hi, w3)):
            for kc in (0, 1):
                dst[("hi", kc)] = wtile([128, HIDDEN], bf, f"{nm}_hi_{kc}",
                                        dhi[kc * 128:(kc + 1) * 128, :])
        for kc in (0, 1):
            w4[("hi", kc)] = wtile([128, ACTION_DIM], bf, f"w4_hi_{kc}",
                                   d_w4_hi[kc * 128:(kc + 1) * 128, :])
        b23 = wtile([4, 2 * 128], bf, "b23", d_b23)
        mask4 = wtile([4, 2 * BPC], bf, "mask4", d_mask)
        noise_sb = wp.tile([ACTION_DIM, T_STEPS * BPC], f32, tag="noise_sb", name="noise_sb")
        nc.gpsimd.dma_start(
            noise_sb.rearrange("p (k c) -> p k c", k=T_STEPS),
            d_noise.rearrange("k p c -> p k c"))
        xb = wtile([ACTION_DIM, T_STEPS], f32, "xb", d_xb)

        sig_bias = wp.tile([128, 1], f32, tag="sig_bias", name="sig_bias")
        nc.vector.memset(sig_bias, -D_S)

        hT = wp.tile([KX + 2, BPC], bf, tag="hT", name="hT")
        nc.gpsimd.dma_start(hT[ACTION_DIM:KX + 2, :], d_state)
        xT = wp.tile([ACTION_DIM, BPC], f32, tag="xT", name="xT")
        nc.gpsimd.dma_start(xT, d_xinit)
        nc.vector.tensor_copy(hT[0:ACTION_DIM, :], xT)

        SIG = mybir.ActivationFunctionType.Sigmoid
        MUL = mybir.AluOpType.mult
        ADD = mybir.AluOpType.add
        MAX = mybir.AluOpType.max
        MIN = mybir.AluOpType.min

        for k in range(nsteps):
            i = T_STEPS - 1 - k
            c1 = float(c1s[i]); c2 = float(c2s[i])
            p1 = float(p1s[i]); p2 = float(p2s[i])

            # early elementwise pieces (only depend on x_k and preloaded noise)
            nz = noise_sb[:, k * BPC:(k + 1) * BPC]
            s2 = sp.tile([ACTION_DIM, BPC], f32, tag="s2", name="s2")
            nc.vector.scalar_tensor_tensor(s2, xT, p2, nz, MUL, ADD)

            # ---- the 3 hidden layers ----
            hprev = None
            for L, (wd, bias_off) in enumerate((
                    (None, None), (w2, 0), (w3, HIDDEN))):
                z = pp.tile([128, 2 * BPC], mybir.dt.float32, tag=f"z{L}", name=f"z{L}")
                if L != 0 and use_b23:
                    boff = (bias_off // HIDDEN) * 128
                    nc.tensor.matmul(z, b23[0:4, boff:boff + 128], mask4, start=True, stop=False)
                for mc in (0, 1):
                    zslice = z[:, mc * BPC:(mc + 1) * BPC]
                    if L == 0:
                        woff = i * 256 + mc * 128
                        nc.tensor.matmul(zslice, w1ext[:, woff:woff + 128], hT, start=True, stop=False)
                        nc.tensor.matmul(zslice, w1x_lo[:, mc * 128:(mc + 1) * 128], hT[0:KX, :], start=False, stop=True)
                    else:
                        for kc in (0, 1):
                            rhs = hprev[:, kc * BPC:(kc + 1) * BPC]
                            nc.tensor.matmul(zslice, wd[("hi", kc)][:, mc * 128:(mc + 1) * 128], rhs,
                                             start=(kc == 0 and not use_b23), stop=(kc == 1))
                # sigmoid pass: s = sigmoid(-(A_S*z + D_S))
                s = ap_.tile([128, 2 * BPC], mybir.dt.float32, tag="s", name="s")
                nc.scalar.activation(s, z, SIG, bias=sig_bias, scale=-A_S)
                # custom completion: h = z*QUAD(t)*CUBIC(t), t = s^2
                wA = ap_.tile([128, 2 * BPC], mybir.dt.float32, tag="wA", name="wA")
                nc.vector._custom_dve(MISH_A, out=wA, in0=s, in1=z, s0=QA, s1=QB, imm2=QC)
                h = ap_.tile([128, 2 * BPC], bf, tag=f"h{L}", name=f"h{L}")
                nc.vector._custom_dve(MISH_B, out=h, in0=s, in1=wA, s0=MA, s1=MB, imm2=MC)
                hprev = h

            # ---- L4: eps psum [16, BPC] ----
            z4 = pp.tile([ACTION_DIM, BPC], mybir.dt.float32, tag="z4", name="z4")
            nc.tensor.matmul(z4, w4[("hi", 0)], hprev[:, 0:BPC], start=True, stop=False)
            nc.tensor.matmul(z4, w4[("hi", 1)], hprev[:, BPC:2 * BPC], start=False, stop=True)

            # ---- x update ----
            pre = sp.tile([ACTION_DIM, BPC], f32, tag="pre", name="pre")
            nc.vector._custom_dve(PREOP, out=pre, in0=z4, in1=xT,
                                  s0=xb[:, i:i + 1], s1=c1, imm2=-c2)
            # x_{k+1} = clip(pre, -1, 1)*p1 + s2: write the bf16 matmul view
            # first (feeds the next step's L1), then the fp32 master.
            nc.vector._custom_dve(CLIPMULADD, out=hT[0:ACTION_DIM, :], in0=pre, in1=s2,
                                  s0=-1.0, s1=1.0, imm2=p1)
            nc.vector._custom_dve(CLIPMULADD, out=xT, in0=pre, in1=s2,
                                  s0=-1.0, s1=1.0, imm2=p1)

        out_f = sp.tile([ACTION_DIM, BPC], f32, tag="out_f", name="out_f")
        nc.vector.tensor_scalar(out_f, xT, -1.0, 1.0, MAX, MIN)
        nc.sync.dma_start(d_out, out_f)

    nc.compile()
    _CACHE[('nc', nsteps, use_b23)] = nc
    return nc


# ---------------------------------------------------------------- host side
def _host_prep(inputs):
    sched = _vp_schedule()
    f64 = np.float64

    W1 = np.asarray(inputs['W1'], np.float32)
    b1 = np.asarray(inputs['b1'], np.float32)
    W2 = np.asarray(inputs['W2'], np.float32)
    b2 = np.asarray(inputs['b2'], np.float32)
    W3 = np.asarray(inputs['W3'], np.float32)
    b3 = np.asarray(inputs['b3'], np.float32)
    W4 = np.asarray(inputs['W4'], np.float32)
    b4 = np.asarray(inputs['b4'], np.float32)

    # time-embedding MLP for all 100 timesteps (host, float64)
    half = TIME_DIM // 2
    freqs = np.exp(np.arange(half, dtype=f64) * (-math.log(10000.0) / (half - 1)))
    ivals = np.arange(T_STEPS, dtype=f64)
    ang = ivals[:, None] * freqs[None, :]
    emb = np.concatenate([np.sin(ang), np.cos(ang)], axis=1)
    t1 = _mish64(emb @ np.asarray(inputs['time_W1'], f64) + np.asarray(inputs['time_b1'], f64))
    temb = t1 @ np.asarray(inputs['time_W2'], f64) + np.asarray(inputs['time_b2'], f64)

    # beta-folded biases
    b2e = b2.astype(f64) + BETA * W2.astype(f64).sum(axis=0)
    b3e = b3.astype(f64) + BETA * W3.astype(f64).sum(axis=0)
    b4e = b4.astype(f64) + BETA * W4.astype(f64).sum(axis=0)

    # contrib[i] = temb[i] @ W1[16:48] + b1   -> flat [1, 100*256]
    contrib = (temb @ W1[16:48].astype(f64) + b1.astype(f64))  # [100, 256]

    def hilo(v):
        v32 = np.asarray(v, np.float32)
        hi = v32.astype(BF16)
        lo = (v32 - hi.astype(np.float32)).astype(BF16)
        return hi, lo

    def pack4(v2d):
        # v2d [G, 256] -> [4, G*128]: rows (hi_a, lo_a, hi_b, lo_b)
        hi, lo = hilo(v2d)
        hi = hi.astype(np.float32); lo = lo.astype(np.float32)
        out = np.stack([hi[:, :128], lo[:, :128], hi[:, 128:], lo[:, 128:]], axis=0)
        return out.reshape(4, -1).astype(BF16)
    b23_hl = pack4(np.stack([b2e, b3e]).astype(np.float32))
    mask4 = np.zeros((4, 2 * BPC), np.float32)
    mask4[0:2, :BPC] = 1.0
    mask4[2:4, BPC:] = 1.0
    mask4 = mask4.astype(BF16)
    w1x = np.concatenate([W1[0:16], W1[48:112]], axis=0)
    w1x_hi, w1x_lo = hilo(w1x)
    c_hi, c_lo = hilo(contrib.astype(np.float32))
    w1ext = np.zeros((KX + 2, T_STEPS * 256), np.float32)
    w1ext[0:KX] = np.tile(np.asarray(w1x_hi, np.float32), (1, T_STEPS))
    w1ext[KX] = np.asarray(c_hi, np.float32).reshape(-1)
    w1ext[KX + 1] = np.asarray(c_lo, np.float32).reshape(-1)
    w1ext = w1ext.astype(BF16)
    w2_hi = np.asarray(W2, np.float32).astype(BF16)
    w3_hi = np.asarray(W3, np.float32).astype(BF16)
    w4_hi = np.asarray(W4, np.float32).astype(BF16)

    # x-update tables
    xb = (-sched['c2'].astype(f64)[None, :] * b4e[:, None]).astype(np.float32)  # [16, 100]

    # per-step noise scaling (fp32, matching the reference ops)
    sig = np.exp(0.5 * sched['logvar']).astype(np.float32)  # [100] by timestep i
    ik = (T_STEPS - 1 - np.arange(T_STEPS))                 # timestep for step k
    scale = sig[ik] * (ik != 0).astype(np.float32)          # [100]
    noise = np.asarray(inputs['noise'], np.float32)
    noise_scaled = noise * scale[:, None, None]

    state = np.asarray(inputs['state'], np.float32)
    x_init = np.asarray(inputs['x_init'], np.float32)

    shared = dict(
        w1ext=w1ext, w1x_lo=w1x_lo, w2_hi=w2_hi,
        w3_hi=w3_hi, w4_hi=w4_hi,
        b23_hl=b23_hl, mask4=mask4,
        xb_t=xb,
    )
    in_maps = []
    for c in range(NCORES):
        sl = slice(c * BPC, (c + 1) * BPC)
        m = dict(shared)
        m['state_t'] = np.ascontiguousarray(
            np.vstack([state[sl].T, np.ones((2, BPC), np.float32)])).astype(BF16)
        m['x_init_t'] = np.ascontiguousarray(x_init[sl].T)
        m['noise_t'] = np.ascontiguousarray(noise_scaled[:, sl, :].transpose(0, 2, 1))
        in_maps.append(m)
    return in_maps


def run(inputs, trace=False, nsteps=T_STEPS):
    use_b23 = bool(max(np.abs(np.asarray(inputs['b2'])).max(),
                       np.abs(np.asarray(inputs['b3'])).max()) > 1e-6)
    nc = _build(nsteps, use_b23)
    in_maps = _host_prep(inputs)
    res = bass_utils.run_bass_kernel_spmd(
        nc, in_maps, core_ids=list(range(NCORES)), trace=trace)
    out = np.empty((BATCH, ACTION_DIM), np.float32)
    for c in range(NCORES):
        out[c * BPC:(c + 1) * BPC] = res.results[c]['out_t'].T
    return out, res


def kernel(**inputs) -> np.ndarray:
    out, _ = run(inputs, trace=False)
    return out



# revision 7
# speedup vs baseline: 947.7546x; 947.7546x over previous
"""Trainium2 Bass kernel for nn_Diffusion_15436112462451.

Strategy: pure data parallelism over the batch (2048 -> 8 cores x 256),
feature-major activations on-chip, and -- the key change vs the unrolled
baseline -- the 100-step denoising loop runs as a single For_i HARDWARE
loop.  The execution environment charges a large fixed cost per STATIC
program instruction (~60us each; measured: an unrolled 2900-instruction
program costs ~183ms while the same work inside a hardware loop is
dominated by true device time ~1ms).  The loop body is ~29 static
instructions; all step-varying quantities are indexed with register-based
dynamic APs (bass.ds) off per-step SBUF tables:

  - noise        [16, 100, BPC] f32, slice [:, ds(k,1), :]
  - temb contrib [128, 100] f32 x2 chunks, column ds(k,1) added to the L1
    PSUM with a per-partition tensor_scalar add (f32 -- exact bias)
  - schedule scalars c1/-c2/p1/p2: [16, 100] f32 tables, column ds(k,1),
    consumed as per-partition scalar APs by the x-update DVE ops

Per step: 12 bf16 matmuls on PE (L1 K=80 x2, L2/L3 4 each, L4 2; biases
b2/b3/b4 ride a mask rank-4 matmul / rank-2 prime only when nonzero --
they are zero in this problem), 3 sigmoid passes on ScalarE, 6 custom-DVE
mish-completion passes (exact-mish quintic in t = sigmoid(-az-d)^2, max
err 6.3e-5), and a 4-op x-update (s2 on Pool, PREOP2/CLIPMA2/bf16-copy on
DVE).  The x iterate is kept in f32 (bf16-only x fails: rel err 5.5e-2 vs
8.5e-4 with f32 master).  The time-embedding MLP is batch-independent and
precomputed on the host into the [100,256] contrib table.
"""
import sys
import math
import re
import numpy as np

for _p in ('/opt/trn_rl_repo', '/root/.axon_site/_ro/trn_rl_repo'):
    if _p not in sys.path:
        sys.path.insert(0, _p)

import ml_dtypes
from contextlib import ExitStack
import concourse.bass as bass
from concourse import bacc
from concourse import mybir, tile, bass_utils, dve_ops
from concourse.dve_spec import Spec, Src0, Src1, C0, C1, C2, sq, maxx, minn

BF16 = ml_dtypes.bfloat16
NCORES = 8
BATCH = 2048
BPC = BATCH // NCORES          # 256 batch rows per core
T_STEPS = 100
STATE_DIM, ACTION_DIM, HIDDEN, TIME_DIM = 64, 16, 256, 32
KX = ACTION_DIM + STATE_DIM    # 80 rows of W1 used for [x; state]

# --- activation fit constants (deg-5 sigma-poly factorization) ---
A_S = 0.9990298806699722
D_S = -0.0005000143935776705
BETA = 4.708088756431602e-05
QA, QB, QC = -0.21302398380145082, 0.6455208072356895, -0.6201860532189531
MA, MB, MC = -0.9194163848641597, 1.5334239721923986, -1.6124382654378613


# ---------------------------------------------------------------- custom ops
def _register_op(name, spec):
    for op in dve_ops.OPS:
        if op.name == name:
            return op
    op = dve_ops.DveOp(name, spec, False, uops_sha={"v3": "?", "v4": "?"})
    dve_ops.OPS.append(op)
    dve_ops.CUSTOM_DVE_SPECS[name] = spec
    dve_ops._SUB_OPCODE_FOR_NAME[name] = (
        dve_ops._CUSTOM_DVE_ROW_BASE + len(dve_ops.OPS) - 1)
    for ver in ("v3", "v4"):
        try:
            op.compile(ver)
        except ValueError as e:
            op.uops_sha[ver] = re.search(
                r'uops_sha\["' + ver + r'"\]="([0-9a-f]+)"', str(e)).group(1)
        op.compile(ver)
    return op


_t = sq(Src0)
MISH_A = _register_op("MISH_A_DIFF15436", Spec(
    body=Src1 * ((_t * C0 + C1) * _t + C2),
    reference=lambda in0, in1, s0, s1, imm2:
        (in1 * ((s0 * in0.astype(np.float64) ** 2 + s1) * in0.astype(np.float64) ** 2 + imm2)).astype(np.float32),
))
_t2 = sq(Src0)
MISH_B = _register_op("MISH_B_DIFF15436", Spec(
    body=Src1 * ((((_t2 + C0) * _t2 + C1) * _t2) + C2),
    reference=lambda in0, in1, s0, s1, imm2:
        (in1 * ((((in0.astype(np.float64) ** 2 + s0) * in0.astype(np.float64) ** 2 + s1) * in0.astype(np.float64) ** 2) + imm2)).astype(np.float32),
))
# pre = z4*C0 + x*C1   (C0 = -c2[k] AP column, C1 = c1[k] AP column)
PREOP2 = _register_op("PREOP2_DIFF15436", Spec(
    body=Src0 * C0 + Src1 * C1,
    reference=lambda in0, in1, s0, s1, imm2:
        (in0 * s0 + in1 * s1).astype(np.float32),
))
# x_new = clip(pre, C1, imm2)*C0 + s2   (C0 = p1[k] AP column; C1=-1, imm2=+1)
CLIPMA2 = _register_op("CLIPMA2_DIFF15436", Spec(
    body=minn(maxx(Src0, C1), C2) * C0 + Src1,
    reference=lambda in0, in1, s0, s1, imm2:
        (np.minimum(np.maximum(in0, s1), imm2) * s0 + in1).astype(np.float32),
))


# ---------------------------------------------------------------- schedule
def _vp_schedule():
    t = np.arange(1, T_STEPS + 1, dtype=np.float64)
    b_max, b_min = 10.0, 0.1
    alpha = np.exp(-b_min / T_STEPS - 0.5 * (b_max - b_min) * (2 * t - 1) / T_STEPS ** 2)
    betas = 1.0 - alpha
    ac = np.cumprod(1.0 - betas)
    ac_prev = np.concatenate([[1.0], ac[:-1]])
    return {
        'c1': np.sqrt(1.0 / ac).astype(np.float32),
        'c2': np.sqrt(1.0 / ac - 1.0).astype(np.float32),
        'p1': (betas * np.sqrt(ac_prev) / (1.0 - ac)).astype(np.float32),
        'p2': ((1.0 - ac_prev) * np.sqrt(1.0 - betas) / (1.0 - ac)).astype(np.float32),
        'logvar': np.log(np.clip(betas * (1.0 - ac_prev) / (1.0 - ac), 1e-20, None)).astype(np.float32),
    }


def _mish64(v):
    return v * np.tanh(np.logaddexp(0.0, v))


# ---------------------------------------------------------------- bass build
_CACHE = {}


def _build(nsteps=T_STEPS, use_b23=False, use_b4=False, repeats=1):
    key = ('nc', nsteps, use_b23, use_b4, repeats)
    if key in _CACHE:
        return _CACHE[key]

    nc = bacc.Bacc("TRN2", target_bir_lowering=False, debug=False, num_devices=NCORES)
    f32 = mybir.dt.float32
    bf = mybir.dt.bfloat16

    def din(name, shape, dt=f32):
        return nc.dram_tensor(name, shape, dt, kind="ExternalInput").ap()

    d_state = din("state_t", [STATE_DIM, BPC], bf)
    d_xinit = din("x_init_t", [ACTION_DIM, BPC])
    d_noise = din("noise_t", [ACTION_DIM, T_STEPS * BPC])
    d_w1 = din("w1_t", [KX, HIDDEN], bf)
    d_w2 = din("w2_t", [128, 2 * HIDDEN], bf)   # (kc, mc) packed
    d_w3 = din("w3_t", [128, 2 * HIDDEN], bf)
    d_w4 = din("w4_t", [128, 2 * ACTION_DIM], bf)
    d_temb = din("temb_t", [128, 2 * T_STEPS])  # chunk0 cols 0:100, chunk1 cols 100:200
    d_sched = din("sched_t", [ACTION_DIM, 4 * T_STEPS])  # p2 | c1 | -c2 | p1
    if use_b23:
        d_b23 = din("b23_hl", [4, 2 * 128], bf)
        d_mask = din("mask4", [4, 2 * BPC], bf)
    if use_b4:
        d_b4 = din("b4_hl", [2, ACTION_DIM], bf)
        d_ones2 = din("ones2", [2, BPC], bf)
    d_out = nc.dram_tensor("out_t", [ACTION_DIM, BPC], f32, kind="ExternalOutput").ap()

    with tile.TileContext(nc) as tc, ExitStack() as ctx:
        wp = ctx.enter_context(tc.tile_pool(name="weights", bufs=1))
        ap_ = ctx.enter_context(tc.tile_pool(name="acts", bufs=1))
        sp = ctx.enter_context(tc.tile_pool(name="small", bufs=1))
        pp = ctx.enter_context(tc.tile_pool(name="psum", bufs=1, space="PSUM"))

        def wtile(shape, dt, nm, src):
            t = wp.tile(shape, dt, tag=nm, name=nm)
            nc.gpsimd.dma_start(t, src)
            return t

        w1 = wtile([KX, HIDDEN], bf, "w1", d_w1)
        w2 = wtile([128, 2 * HIDDEN], bf, "w2", d_w2)
        w3 = wtile([128, 2 * HIDDEN], bf, "w3", d_w3)
        w4 = wtile([128, 2 * ACTION_DIM], bf, "w4", d_w4)
        temb = wtile([128, 2 * T_STEPS], f32, "temb", d_temb)
        sched = wtile([ACTION_DIM, 4 * T_STEPS], f32, "sched", d_sched)
        if use_b23:
            b23 = wtile([4, 2 * 128], bf, "b23", d_b23)
            mask4 = wtile([4, 2 * BPC], bf, "mask4", d_mask)
        if use_b4:
            b4hl = wtile([2, ACTION_DIM], bf, "b4hl", d_b4)
            ones2 = wtile([2, BPC], bf, "ones2", d_ones2)
        noise_sb = wp.tile([ACTION_DIM, T_STEPS * BPC], f32, tag="noise_sb", name="noise_sb")
        nc.gpsimd.dma_start(noise_sb, d_noise)
        noise3 = noise_sb.rearrange("p (k c) -> p k c", k=T_STEPS)

        sig_bias = wp.tile([128, 1], f32, tag="sig_bias", name="sig_bias")
        nc.vector.memset(sig_bias, -D_S)

        hT = wp.tile([KX, BPC], bf, tag="hT", name="hT")
        nc.gpsimd.dma_start(hT[ACTION_DIM:KX, :], d_state)
        xT = wp.tile([ACTION_DIM, BPC], f32, tag="xT", name="xT")
        nc.gpsimd.dma_start(xT, d_xinit)
        nc.vector.tensor_copy(hT[0:ACTION_DIM, :], xT)

        # persistent activation / psum tiles (static addresses inside the loop)
        h1 = ap_.tile([128, 2 * BPC], bf, tag="h1", name="h1")
        h2 = ap_.tile([128, 2 * BPC], bf, tag="h2", name="h2")
        h3 = ap_.tile([128, 2 * BPC], bf, tag="h3", name="h3")
        s_t = ap_.tile([128, 2 * BPC], f32, tag="s_t", name="s_t")
        wA = ap_.tile([128, 2 * BPC], f32, tag="wA", name="wA")
        s2 = sp.tile([ACTION_DIM, BPC], f32, tag="s2", name="s2")
        pre = sp.tile([ACTION_DIM, BPC], f32, tag="pre", name="pre")
        z1 = pp.tile([128, 2 * BPC], f32, tag="z1", name="z1")
        z2 = pp.tile([128, 2 * BPC], f32, tag="z2", name="z2")
        z3 = pp.tile([128, 2 * BPC], f32, tag="z3", name="z3")
        z4 = pp.tile([ACTION_DIM, BPC], f32, tag="z4", name="z4")

        SIG = mybir.ActivationFunctionType.Sigmoid
        MUL = mybir.AluOpType.mult
        ADD = mybir.AluOpType.add
        MAX = mybir.AluOpType.max
        MIN = mybir.AluOpType.min

        def mish(z, h):
            nc.scalar.activation(s_t, z, SIG, bias=sig_bias, scale=-A_S)
            nc.vector._custom_dve(MISH_A, out=wA, in0=s_t, in1=z, s0=QA, s1=QB, imm2=QC)
            nc.vector._custom_dve(MISH_B, out=h, in0=s_t, in1=wA, s0=MA, s1=MB, imm2=MC)

        import contextlib
        rep_cm = tc.For_i(0, repeats) if repeats > 1 else contextlib.nullcontext()
        with rep_cm, tc.For_i(0, nsteps) as k:
            kc1 = bass.ds(k, 1)

            # s2 = p2[k]*x + noise_k
            nc.vector.scalar_tensor_tensor(
                s2.rearrange("p (a c) -> p a c", a=1), xT.rearrange("p (a c) -> p a c", a=1),
                sched[:, 0:T_STEPS][:, kc1], noise3[:, kc1, :], MUL, ADD)

            # ---- L1: z1 = W1x^T [x; state]  + temb[k] ----
            for mc in (0, 1):
                nc.tensor.matmul(z1[:, mc * BPC:(mc + 1) * BPC],
                                 w1[:, mc * 128:(mc + 1) * 128], hT,
                                 start=True, stop=True)
                nc.vector.tensor_scalar_add(
                    z1[:, mc * BPC:(mc + 1) * BPC],
                    z1[:, mc * BPC:(mc + 1) * BPC],
                    temb[:, mc * T_STEPS:(mc + 1) * T_STEPS][:, kc1])
            mish(z1, h1)

            # ---- L2 / L3 ----
            for wd, hin, zt, hout, boff in ((w2, h1, z2, h2, 0), (w3, h2, z3, h3, 128)):
                if use_b23:
                    nc.tensor.matmul(zt, b23[0:4, boff:boff + 128], mask4,
                                     start=True, stop=False)
                for mc in (0, 1):
                    zslice = zt[:, mc * BPC:(mc + 1) * BPC]
                    for kc in (0, 1):
                        nc.tensor.matmul(
                            zslice,
                            wd[:, kc * HIDDEN + mc * 128:kc * HIDDEN + (mc + 1) * 128],
                            hin[:, kc * BPC:(kc + 1) * BPC],
                            start=(kc == 0 and not use_b23), stop=(kc == 1))
                mish(zt, hout)

            # ---- L4: eps psum [16, BPC] ----
            if use_b4:
                nc.tensor.matmul(z4, b4hl, ones2, start=True, stop=False)
            nc.tensor.matmul(z4, w4[:, 0:ACTION_DIM], h3[:, 0:BPC],
                             start=not use_b4, stop=False)
            nc.tensor.matmul(z4, w4[:, ACTION_DIM:2 * ACTION_DIM], h3[:, BPC:2 * BPC],
                             start=False, stop=True)

            # ---- x update ----
            nc.vector._custom_dve(PREOP2, out=pre, in0=z4, in1=xT,
                                  s0=sched[:, 2 * T_STEPS:3 * T_STEPS][:, kc1],
                                  s1=sched[:, T_STEPS:2 * T_STEPS][:, kc1])
            nc.vector._custom_dve(CLIPMA2, out=xT, in0=pre, in1=s2,
                                  s0=sched[:, 3 * T_STEPS:4 * T_STEPS][:, kc1],
                                  s1=-1.0, imm2=1.0)
            nc.vector.tensor_copy(hT[0:ACTION_DIM, :], xT)

        out_f = sp.tile([ACTION_DIM, BPC], f32, tag="out_f", name="out_f")
        nc.vector.tensor_scalar(out_f, xT, -1.0, 1.0, MAX, MIN)
        nc.sync.dma_start(d_out, out_f)

    nc.compile()
    _CACHE[key] = nc
    return nc


# ---------------------------------------------------------------- host side
def _host_prep(inputs):
    sched = _vp_schedule()
    f64 = np.float64

    W1 = np.asarray(inputs['W1'], np.float32)
    b1 = np.asarray(inputs['b1'], np.float32)
    W2 = np.asarray(inputs['W2'], np.float32)
    b2 = np.asarray(inputs['b2'], np.float32)
    W3 = np.asarray(inputs['W3'], np.float32)
    b3 = np.asarray(inputs['b3'], np.float32)
    W4 = np.asarray(inputs['W4'], np.float32)
    b4 = np.asarray(inputs['b4'], np.float32)

    # time-embedding MLP for all 100 timesteps (host, float64)
    half = TIME_DIM // 2
    freqs = np.exp(np.arange(half, dtype=f64) * (-math.log(10000.0) / (half - 1)))
    ivals = np.arange(T_STEPS, dtype=f64)
    ang = ivals[:, None] * freqs[None, :]
    emb = np.concatenate([np.sin(ang), np.cos(ang)], axis=1)
    t1 = _mish64(emb @ np.asarray(inputs['time_W1'], f64) + np.asarray(inputs['time_b1'], f64))
    temb = t1 @ np.asarray(inputs['time_W2'], f64) + np.asarray(inputs['time_b2'], f64)

    # beta-folded biases (the quintic mish fit is exact-mish + BETA; fold the
    # constant BETA into the next layer's bias)
    b2e = (b2.astype(f64) + BETA * W2.astype(f64).sum(axis=0)).astype(np.float32)
    b3e = (b3.astype(f64) + BETA * W3.astype(f64).sum(axis=0)).astype(np.float32)
    b4e = (b4.astype(f64) + BETA * W4.astype(f64).sum(axis=0)).astype(np.float32)

    # contrib[i] = temb[i] @ W1[16:48] + b1  -> per-step L1 bias, f32 exact
    contrib = (temb @ W1[16:48].astype(f64) + b1.astype(f64)).astype(np.float32)  # [100, 256] by timestep i

    ik = T_STEPS - 1 - np.arange(T_STEPS)   # timestep for loop iteration k
    # temb table by k: [128, 2*100] (chunk0 | chunk1)
    ck = contrib[ik]                        # [100, 256] by k
    temb_t = np.concatenate([ck[:, 0:128].T, ck[:, 128:256].T], axis=1).astype(np.float32)

    # schedule tables by k, replicated over the 16 feature partitions:
    # [16, 4*100] = p2 | c1 | -c2 | p1
    c1k = sched['c1'][ik]; c2k = sched['c2'][ik]
    p1k = sched['p1'][ik]; p2k = sched['p2'][ik]
    sched_t = np.concatenate([
        np.tile(p2k, (ACTION_DIM, 1)),
        np.tile(c1k, (ACTION_DIM, 1)),
        np.tile(-c2k, (ACTION_DIM, 1)),
        np.tile(p1k, (ACTION_DIM, 1)),
    ], axis=1).astype(np.float32)

    def hilo(v):
        v32 = np.asarray(v, np.float32)
        hi = v32.astype(BF16)
        lo = (v32 - hi.astype(np.float32)).astype(BF16)
        return hi, lo

    w1x = np.concatenate([W1[0:16], W1[48:112]], axis=0)
    w1_t = w1x.astype(BF16)
    # W2/W3 packed [(kc), 128, (mc)] -> [128, 2*256]: cols kc*256+mc*128
    def pack_w(W):
        out = np.zeros((128, 2 * HIDDEN), np.float32)
        for kc in (0, 1):
            for mc in (0, 1):
                out[:, kc * HIDDEN + mc * 128:kc * HIDDEN + (mc + 1) * 128] = \
                    W[kc * 128:(kc + 1) * 128, mc * 128:(mc + 1) * 128]
        return out.astype(BF16)
    w2_t = pack_w(W2)
    w3_t = pack_w(W3)
    w4_t = np.concatenate([W4[0:128], W4[128:256]], axis=1).astype(BF16)

    use_b23 = bool(max(np.abs(b2e).max(), np.abs(b3e).max()) > 1e-7)
    use_b4 = bool(np.abs(b4e).max() > 1e-7)

    shared = dict(
        w1_t=w1_t, w2_t=w2_t, w3_t=w3_t, w4_t=w4_t,
        temb_t=temb_t, sched_t=sched_t,
    )
    if use_b23:
        def pack4(v2d):
            hi, lo = hilo(v2d)
            hi = hi.astype(np.float32); lo = lo.astype(np.float32)
            out = np.stack([hi[:, :128], lo[:, :128], hi[:, 128:], lo[:, 128:]], axis=0)
            return out.reshape(4, -1).astype(BF16)
        shared['b23_hl'] = pack4(np.stack([b2e, b3e]))
        mask4 = np.zeros((4, 2 * BPC), np.float32)
        mask4[0:2, :BPC] = 1.0
        mask4[2:4, BPC:] = 1.0
        shared['mask4'] = mask4.astype(BF16)
    if use_b4:
        hi, lo = hilo(b4e)
        shared['b4_hl'] = np.stack([hi, lo]).astype(BF16)
        shared['ones2'] = np.ones((2, BPC), np.float32).astype(BF16)

    # per-step noise scaling (fp32, matching the reference ops)
    sig = np.exp(0.5 * sched['logvar']).astype(np.float32)  # [100] by timestep i
    scale = sig[ik] * (ik != 0).astype(np.float32)          # [100] by k
    noise = np.asarray(inputs['noise'], np.float32)
    noise_scaled = noise * scale[:, None, None]             # [100, B, 16]

    state = np.asarray(inputs['state'], np.float32)
    x_init = np.asarray(inputs['x_init'], np.float32)

    in_maps = []
    for c in range(NCORES):
        sl = slice(c * BPC, (c + 1) * BPC)
        m = dict(shared)
        m['state_t'] = np.ascontiguousarray(state[sl].T).astype(BF16)
        m['x_init_t'] = np.ascontiguousarray(x_init[sl].T)
        # noise3[p, k, c] = noise_scaled[k, batch c, feature p]
        m['noise_t'] = np.ascontiguousarray(
            noise_scaled[:, sl, :].transpose(2, 0, 1).reshape(ACTION_DIM, T_STEPS * BPC))
        in_maps.append(m)
    return in_maps, use_b23, use_b4


def run(inputs, trace=False, nsteps=T_STEPS):
    in_maps, use_b23, use_b4 = _host_prep(inputs)
    nc = _build(nsteps, use_b23, use_b4)
    res = bass_utils.run_bass_kernel_spmd(
        nc, in_maps, core_ids=list(range(NCORES)), trace=trace)
    out = np.empty((BATCH, ACTION_DIM), np.float32)
    for c in range(NCORES):
        out[c * BPC:(c + 1) * BPC] = res.results[c]['out_t'].T
    return out, res


def kernel(**inputs) -> np.ndarray:
    out, _ = run(inputs, trace=False)
    return out
